# revision 4
# baseline (speedup 1.0000x reference)
"""Trainium2 Bass kernel for nn_BaseDependentAttentionLayer (GNN edge attention).

v2 redesign vs baseline:
  - Global LPT balancing of origin nodes across all 80 windows (8 cores x 10)
    so every window carries ~2000 edges -> T=16 tiles (was 17), less padding.
  - Single index tensor addresses both k_full and v_full (same row ids).
  - Split AllGather into k and v collectives; k AG overlaps v/q compute.
  - PSUM-direct elementwise: DVE reads qg/b/sc straight from PSUM (bf16 PSUM
    matmul outputs), eliminating the big ACT PSUM->SBUF copies.
  - Denominator via per-tile transposed scores (PE transpose + ACT exp ->
    [128e, 8h]) and an 8-col scatter matmul, replacing the [128,512]
    head-replicated denominator matmul + b_sb copy.
  - One-hot scatter matrices built on the Pool engine (is_equal vs iota).
  - Per-window fused epilogue: softmax divide via ACT scaled-copies from
    PSUM, transpose, MLP matmuls and residual add, streaming to out.
  - x rows stay SBUF-resident for the residual (no HBM re-read).
"""

import sys

sys.path.insert(0, "/opt/trn_rl_repo")

import numpy as np
import ml_dtypes

bf16 = ml_dtypes.bfloat16

N, E, D, H = 10000, 160000, 512, 8
HD = D // H
SCALE = HD**-0.5
NCORES = 8
W = 10  # windows per core
WIN = 128  # origin slots per window
ROWS = W * WIN  # 1280 node rows per core (incl. pad slots)
NWIN = NCORES * W
ET = 128  # edges per tile
EPS_LN = 1e-5
EPS_DEN = 1e-16


def _host_prep(origin, dest, ew):
    """Globally balance origins over 80 windows; build per-core edge tables."""
    import heapq

    deg = np.bincount(origin, minlength=N).astype(np.int64)
    order = np.argsort(-deg, kind="stable")
    heap = [(0, wi) for wi in range(NWIN)]
    heapq.heapify(heap)
    wsize = np.zeros(NWIN, np.int64)
    wload = np.zeros(NWIN, np.int64)
    wof = np.zeros(N, np.int32)
    slot = np.zeros(N, np.int32)
    for n in order:
        load, wi = heapq.heappop(heap)
        wof[n] = wi
        slot[n] = wsize[wi]
        wsize[wi] += 1
        wload[wi] += deg[n]
        if wsize[wi] < WIN:
            heapq.heappush(heap, (load + deg[n], wi))
    T = max(1, int(np.ceil(wload.max() / ET)))
    NB = W * T * ET

    # global k/v row of node n
    wslot = (wof // W).astype(np.int64) * ROWS + (wof % W).astype(np.int64) * WIN + slot

    # node id at each row (or -1)
    perm = np.full(NCORES * ROWS, -1, np.int64)
    perm[wslot] = np.arange(N)

    wo = wof[origin]  # window of each edge
    eorder = np.argsort(wo, kind="stable")
    wcnt = np.bincount(wo, minlength=NWIN)
    woff = np.zeros(NWIN + 1, np.int64)
    np.cumsum(wcnt, out=woff[1:])

    percore = []
    for r in range(NCORES):
        dk = np.zeros((W, T * ET), np.int16)
        ol = np.full((W, T * ET), 255.0, np.float32)
        et = np.zeros((W, T * ET, H), np.float32)
        st = np.zeros((W, WIN, T * ET), np.float32)
        for w in range(W):
            wi = r * W + w
            es = eorder[woff[wi] : woff[wi + 1]]
            cnt = len(es)
            assert cnt <= T * ET
            sl = slot[origin[es]]
            dk[w, :cnt] = wslot[dest[es]].astype(np.int16)
            ol[w, :cnt] = sl.astype(np.float32)
            et[w, :cnt] = ew[es]
            st[w][sl, np.arange(cnt)] = 1.0
        percore.append(dict(dk=dk, ol=ol, et=et, st=st))
    return percore, T, wslot, perm


def _wrap_idx(idx_flat):
    """int16 [n] -> wrapped [128, n/16] layout for dma_gather."""
    w = idx_flat.reshape(-1, 16).T
    return np.tile(w, (8, 1)).astype(np.int16)


def _build_program(T, mock_ag=False, opts=()):
    opts = set(opts)
    import concourse.bass as bass
    import concourse.bacc as bacc
    import concourse.mybir as mybir
    import concourse.tile as tile

    dt = mybir.dt
    Alu = mybir.AluOpType
    Act = mybir.ActivationFunctionType

    NB = W * T * ET
    HT0 = (T + 1) // 2  # tiles in first half-window gather
    HT1 = T - HT0

    nc = bacc.Bacc(
        "TRN2", target_bir_lowering=False, debug=False, num_devices=NCORES
    )

    # ---------------- I/O ----------------
    xs_t = nc.dram_tensor("xs", [ROWS, D], dt.float32, kind="ExternalInput")
    wq_t = nc.dram_tensor("wq", [128, 4, D], dt.bfloat16, kind="ExternalInput")
    wk_t = nc.dram_tensor("wk", [128, 4, D], dt.bfloat16, kind="ExternalInput")
    wv_t = nc.dram_tensor("wv", [128, 4, D], dt.bfloat16, kind="ExternalInput")
    w12_t = nc.dram_tensor("w12", [128, 4, D], dt.bfloat16, kind="ExternalInput")
    bias_t = nc.dram_tensor("bias", [1, 4, D], dt.bfloat16, kind="ExternalInput")
    hmask_t = nc.dram_tensor("hmask", [128, 4, H], dt.bfloat16, kind="ExternalInput")
    m1_t = nc.dram_tensor("m1", [H, D], dt.bfloat16, kind="ExternalInput")
    ones_t = nc.dram_tensor("ones1", [1, 128], dt.bfloat16, kind="ExternalInput")
    ident_t = nc.dram_tensor("ident", [128, 128], dt.bfloat16, kind="ExternalInput")
    id32_t = nc.dram_tensor("id32", [128, 128], dt.float32, kind="ExternalInput")
    iotaF_t = nc.dram_tensor("iotaF", [128, 128], dt.bfloat16, kind="ExternalInput")
    dkw_t = nc.dram_tensor("dkw", [128, NB // 16], dt.int16, kind="ExternalInput")
    oloc_t = nc.dram_tensor("oloc", [128, W * T], dt.float32, kind="ExternalInput")
    st_t = nc.dram_tensor("st", [128, NB], dt.bfloat16, kind="ExternalInput")
    ewt_t = nc.dram_tensor("ewt", [H, NB], dt.bfloat16, kind="ExternalInput")
    out_t = nc.dram_tensor("out", [ROWS, D], dt.float32, kind="ExternalOutput")

    with tile.TileContext(nc) as tc:
        with (
            tc.tile_pool(name="const", bufs=1) as cpool,
            tc.tile_pool(name="persist", bufs=1) as ppool,
            tc.tile_pool(name="dram", bufs=1, space="DRAM") as dpool,
        ):
            # constants
            wq = cpool.tile([128, 4, D], dt.bfloat16)
            wk = cpool.tile([128, 4, D], dt.bfloat16)
            wv = cpool.tile([128, 4, D], dt.bfloat16)
            w12 = cpool.tile([128, 4, D], dt.bfloat16)
            biases = cpool.tile([1, 4, D], dt.bfloat16)
            hmask = cpool.tile([128, 4, H], dt.bfloat16)
            m1 = cpool.tile([H, D], dt.bfloat16)
            ones1 = cpool.tile([1, 128], dt.bfloat16)
            ident = cpool.tile([128, 128], dt.bfloat16)
            id32 = cpool.tile([128, 128], dt.float32)
            iotaF = cpool.tile([128, 128], dt.bfloat16)
            dkw = cpool.tile([128, NB // 16], dt.int16)
            oloc = cpool.tile([128, W * T], dt.float32)

            # persistent activations
            xs_sb = ppool.tile([128, W, D], dt.float32)
            zts = ppool.tile([128, W, 4, 128], dt.bfloat16)
            q_sb = ppool.tile([128, W, D], dt.bfloat16)

            # collective buffers
            k_in = dpool.tile([ROWS, D], dt.bfloat16)
            v_in = dpool.tile([ROWS, D], dt.bfloat16)
            k_full = dpool.tile([ROWS * NCORES, D], dt.bfloat16)
            v_full = dpool.tile([ROWS * NCORES, D], dt.bfloat16)

            # ---------------- Phase A: LN + K (loop1), V (loop2), Q (loop3) --
            with (
                tc.tile_pool(name="pA", bufs=3) as pa,
                tc.tile_pool(name="psA", bufs=2, space="PSUM") as psa,
            ):
                for g in range(W):
                    nc.sync.dma_start(
                        xs_sb[:, g, :], xs_t.ap()[g * 128 : (g + 1) * 128, :]
                    )
                for tl, tn in [
                    (ident, ident_t), (wk, wk_t), (wv, wv_t),
                    (biases, bias_t), (ones1, ones_t), (wq, wq_t),
                    (hmask, hmask_t), (m1, m1_t), (id32, id32_t),
                    (iotaF, iotaF_t), (dkw, dkw_t), (oloc, oloc_t),
                    (w12, w12_t),
                ]:
                    nc.sync.dma_start(tl[:], tn.ap())
                for g in range(W):
                    lo = g * 128
                    xg = xs_sb[:, g, :]
                    musum = pa.tile([128, 1], dt.float32, tag="musum")
                    nc.vector.tensor_reduce(
                        musum[:], xg, mybir.AxisListType.X, Alu.add
                    )
                    mu = pa.tile([128, 1], dt.float32, tag="mu")
                    nc.vector.tensor_scalar_mul(mu[:], musum[:], 1.0 / D)
                    sq = pa.tile([128, D], dt.float32, tag="sq")
                    vs = pa.tile([128, 1], dt.float32, tag="vs")
                    nc.vector.scalar_tensor_tensor(
                        sq[:], xg, 1.0, xg, Alu.bypass, Alu.mult, accum_out=vs[:]
                    )
                    mu2 = pa.tile([128, 1], dt.float32, tag="mu2")
                    nc.vector.tensor_tensor(mu2[:], mu[:], mu[:], Alu.mult)
                    vr = pa.tile([128, 1], dt.float32, tag="vr")
                    nc.vector.scalar_tensor_tensor(
                        vr[:], vs[:], 1.0 / D, mu2[:], Alu.mult, Alu.subtract
                    )
                    vr2 = pa.tile([128, 1], dt.float32, tag="vr2")
                    nc.vector.tensor_scalar(vr2[:], vr[:], EPS_LN, None, Alu.add)
                    sd = pa.tile([128, 1], dt.float32, tag="sd")
                    nc.scalar.sqrt(sd[:], vr2[:])
                    rstd = pa.tile([128, 1], dt.float32, tag="rstd")
                    nc.vector.reciprocal(rstd[:], sd[:])
                    z = pa.tile([128, D], dt.bfloat16, tag="z")
                    nc.vector.tensor_scalar(
                        z[:], xg, mu[:], rstd[:], Alu.subtract, Alu.mult
                    )
                    zT_ps = psa.tile([128, 4, 128], dt.bfloat16, tag="zT_ps")
                    for c in range(4):
                        nc.tensor.transpose(
                            zT_ps[:, c, :], z[:, c * 128 : (c + 1) * 128], ident[:]
                        )
                    nc.scalar.copy(zts[:, g, :, :], zT_ps[:])
                    # k and v share the z-chunk stationaries; both
                    # feed the early collectives
                    kps = psa.tile([128, D], dt.float32, tag="k_ps")
                    vps = psa.tile([128, D], dt.float32, tag="v_ps")
                    for c in range(4):
                        nc.tensor.matmul(
                            kps[:], zts[:, g, c, :], wk[:, c, :],
                            start=(c == 0), stop=False,
                        )
                        nc.tensor.matmul(
                            vps[:], zts[:, g, c, :], wv[:, c, :],
                            start=(c == 0), stop=False,
                        )
                    nc.tensor.matmul(
                        kps[:], ones1[:], biases[:, 1, :], start=False, stop=True
                    )
                    nc.tensor.matmul(
                        vps[:], ones1[:], biases[:, 2, :], start=False, stop=True
                    )
                    kt = pa.tile([128, D], dt.bfloat16, tag="kvt")
                    nc.scalar.copy(kt[:], kps[:])
                    nc.sync.dma_start(k_in[lo : lo + 128, :], kt[:])
                    vt = pa.tile([128, D], dt.bfloat16, tag="kvt")
                    nc.scalar.copy(vt[:], vps[:])
                    nc.sync.dma_start(v_in[lo : lo + 128, :], vt[:])

                if mock_ag:
                    nc.sync.dma_start(k_full[0:ROWS, :], k_in[:])
                else:
                    nc.gpsimd.collective_compute(
                        "AllGather", Alu.bypass,
                        replica_groups=[list(range(NCORES))],
                        ins=[k_in.opt()], outs=[k_full.opt()],
                    )

                if mock_ag:
                    nc.sync.dma_start(v_full[0:ROWS, :], v_in[:])
                else:
                    nc.gpsimd.collective_compute(
                        "AllGather", Alu.bypass,
                        replica_groups=[list(range(NCORES))],
                        ins=[v_in.opt()], outs=[v_full.opt()],
                    )

                for g in range(W):
                    qps = psa.tile([128, D], dt.float32, tag="k_ps")
                    for c in range(4):
                        nc.tensor.matmul(
                            qps[:], zts[:, g, c, :], wq[:, c, :],
                            start=(c == 0), stop=False,
                        )
                    nc.tensor.matmul(
                        qps[:], ones1[:], biases[:, 0, :], start=False, stop=True
                    )
                    nc.scalar.copy(q_sb[:, g, :], qps[:])

            # ---------------- Phase B: edge loop + fused epilogue ------------
            HALves = [list(range(HT0)), list(range(HT0, T))]
            with (
                tc.tile_pool(name="pB", bufs=2) as pb,
                tc.tile_pool(name="psQG",
                             bufs=1 if ("qg1" in opts or "qg4" in opts) else 2,
                             space="PSUM") as psqg,
                tc.tile_pool(name="psSC", bufs=2 if "sc2" in opts else 1,
                             space="PSUM") as pssc,
                tc.tile_pool(name="psWT", bufs=1, space="PSUM") as pswt,
                tc.tile_pool(name="psB2", bufs=1, space="PSUM") as psb2,
                tc.tile_pool(name="psAcc", bufs=1, space="PSUM") as psacc,
            ):
                def emit_epilogue(unnorm, den, w):
                    # ---- fused epilogue: divide, transpose, MLP, residual ----
                    den8 = pb.tile([128, H], dt.float32, tag="den8")
                    nc.vector.tensor_scalar(
                        den8[:], den[:], EPS_DEN, None, Alu.add
                    )
                    rec8 = pb.tile([128, H], dt.float32, tag="rec8")
                    nc.vector.reciprocal(rec8[:], den8[:])
                    vals = pb.tile([128, D], dt.bfloat16, tag="vals")
                    for h in range(H):
                        nc.scalar.activation(
                            vals[:, h * HD : (h + 1) * HD],
                            unnorm[:, h * HD : (h + 1) * HD],
                            Act.Copy, scale=rec8[:, h : h + 1],
                        )
                    vt_sb = pb.tile([128, 4, 128], dt.bfloat16, tag="vt_sb")
                    if "qg4" in opts:
                        vt_ps = psqg.tile([128, 4, 512], dt.bfloat16, tag="qg")
                        for c in range(4):
                            nc.tensor.transpose(
                                vt_ps[:, c, :128],
                                vals[:, c * 128 : (c + 1) * 128],
                                ident[:],
                            )
                        nc.scalar.copy(vt_sb[:], vt_ps[:, :, :128])
                    else:
                        for cp in range(2):
                            vt_ps = psqg.tile(
                                [128, 2, 512], dt.bfloat16, tag="qg"
                            )
                            for c2 in range(2):
                                c = cp * 2 + c2
                                nc.tensor.transpose(
                                    vt_ps[:, c2, :128],
                                    vals[:, c * 128 : (c + 1) * 128],
                                    ident[:],
                                )
                            nc.scalar.copy(
                                vt_sb[:, 2 * cp : 2 * cp + 2, :],
                                vt_ps[:, :, :128]
                            )
                    if "scmlp" in opts:
                        mlp = pssc.tile([128, D], dt.float32, tag="sc")
                    else:
                        mlp = pssc.tile([128, D], dt.float32, tag="mlp")
                    for c in range(4):
                        nc.tensor.matmul(
                            mlp[:], vt_sb[:, c, :], w12[:, c, :],
                            start=(c == 0), stop=False,
                        )
                    nc.tensor.matmul(
                        mlp[:], ones1[:], biases[:, 3, :], start=False, stop=True
                    )
                    og = pb.tile([128, D], dt.float32, tag="og")
                    nc.vector.tensor_tensor(
                        og[:], mlp[:], xs_sb[:, w, :], Alu.add
                    )
                    nc.sync.dma_start(
                        out_t.ap()[w * 128 : (w + 1) * 128, :], og[:]
                    )

                pend = None
                for w in range(W):
                    halves = []
                    for hf, tl in enumerate(HALves):
                        nht = len(tl)
                        ni = nht * ET
                        c0 = (w * T + tl[0]) * ET // 16
                        kT = pb.tile([128, 4, ni], dt.bfloat16, tag=f"kT{ni}", bufs=3)
                        nc.gpsimd.dma_gather(
                            out_ap=kT[:], in_ap=k_full[:],
                            idxs_ap=dkw[:, c0 : c0 + ni // 16],
                            num_idxs=ni, num_idxs_reg=ni, elem_size=D,
                            transpose=True, single_packet=False,
                        )
                        vG = pb.tile([128, nht, D], dt.bfloat16, tag=f"vG{ni}", bufs=3)
                        nc.gpsimd.dma_gather(
                            out_ap=vG[:], in_ap=v_full[:],
                            idxs_ap=dkw[:, c0 : c0 + ni // 16],
                            num_idxs=ni, num_idxs_reg=ni, elem_size=D,
                            single_packet=False,
                        )
                        halves.append((kT, vG))

                    stw = pb.tile([128, T * ET], dt.bfloat16, tag="stw")
                    nc.sync.dma_start(
                        stw[:], st_t.ap()[:, w * T * ET : (w + 1) * T * ET]
                    )
                    ewtw = pb.tile([H, T * ET], dt.bfloat16, tag="ewtw")
                    nc.sync.dma_start(
                        ewtw[:], ewt_t.ap()[:, w * T * ET : (w + 1) * T * ET]
                    )
                    # one-hot scatter matrices for the whole window (Pool),
                    # hoisted off the per-tile critical path
                    sgw = pb.tile([128, T, 128], dt.bfloat16, tag="sgw")
                    for tt in range(T):
                        nc.gpsimd.tensor_scalar(
                            sgw[:, tt, :], iotaF[:],
                            oloc[:, w * T + tt : w * T + tt + 1],
                            None, Alu.is_equal,
                        )

                    acc2 = "epidefer" in opts or "un2" in opts
                    unnorm = psacc.tile(
                        [128, D], dt.float32, tag="unnorm",
                        bufs=2 if acc2 else 1,
                    )
                    den = psacc.tile(
                        [128, H], dt.float32, tag="den",
                        bufs=2 if "epidefer" in opts else 1,
                    )

                    blocks = []
                    for hf, tl in enumerate(HALves):
                        nht = len(tl)
                        for b0 in range(0, nht, 4):
                            bt = min(4, nht - b0)
                            blocks.append((hf, b0, bt, tl[0]))

                    if "staged" in opts:
                        # software-pipelined emission: stage the whole
                        # window so each engine's in-order stream overlaps
                        # blocks instead of serializing the 8-hop chain
                        kqs, wss, ewss, ewsTs, wvgs = {}, {}, {}, {}, {}
                        for bi, (hf, b0, bt, t0) in enumerate(blocks):
                            kT, vG = halves[hf]
                            EB = bt * ET
                            ecol = (t0 + b0) * ET
                            kq = pb.tile([128, 4, 512], dt.bfloat16,
                                         tag="kq", bufs=4)
                            for cp in range(2):
                                qg = psqg.tile(
                                    [128, 2, 512], dt.bfloat16, tag="qg"
                                )
                                for c2 in range(2):
                                    c = cp * 2 + c2
                                    nc.tensor.matmul(
                                        qg[:, c2, :EB],
                                        q_sb[:, w, c * 128 : (c + 1) * 128],
                                        stw[:, ecol : ecol + EB],
                                        is_transpose=True,
                                    )
                                nc.vector.tensor_tensor(
                                    kq[:, 2 * cp : 2 * cp + 2, :EB],
                                    kT[:, 2 * cp : 2 * cp + 2,
                                       b0 * ET : b0 * ET + EB],
                                    qg[:, :, :EB],
                                    Alu.mult,
                                )
                            kqs[bi] = kq
                        for bi, (hf, b0, bt, t0) in enumerate(blocks):
                            EB = bt * ET
                            ecol = (t0 + b0) * ET
                            kq = kqs[bi]
                            if "scmlp" in opts:
                                sc_ps = pssc.tile(
                                    [128, 512], dt.float32, tag="sc"
                                )
                            else:
                                sc_ps = pssc.tile([H, 512], dt.float32, tag="sc")
                            for c in range(4):
                                nc.tensor.matmul(
                                    sc_ps[:H, :EB], hmask[:, c, :],
                                    kq[:, c, :EB],
                                    start=(c == 0), stop=(c == 3),
                                )
                            ws = pb.tile([H, 512], dt.float32, tag="ws",
                                         bufs=5)
                            nc.vector.tensor_tensor(
                                ws[:, :EB], sc_ps[:H, :EB],
                                ewtw[:, ecol : ecol + EB], Alu.mult,
                            )
                            ews = pb.tile([H, 512], dt.bfloat16, tag="ews",
                                          bufs=5)
                            nc.scalar.activation(
                                ews[:, :EB], ws[:, :EB], Act.Exp
                            )
                            wss[bi], ewss[bi] = ws, ews
                        if "ewstbg" not in opts:
                            for bi, (hf, b0, bt, t0) in enumerate(blocks):
                                ewsT = pb.tile([128, 4, H], dt.bfloat16,
                                               tag="ewsT", bufs=5)
                                for t in range(bt):
                                    wsT = pswt.tile(
                                        [128, H], dt.float32, tag="wsT"
                                    )
                                    nc.tensor.matmul(
                                        wsT[:],
                                        wss[bi][:, t * ET : (t + 1) * ET],
                                        id32[:H, :H], is_transpose=True,
                                    )
                                    nc.scalar.activation(
                                        ewsT[:, t, :], wsT[:], Act.Exp
                                    )
                                ewsTs[bi] = ewsT
                        for bi, (hf, b0, bt, t0) in enumerate(blocks):
                            kT, vG = halves[hf]
                            if "ewstbg" in opts:
                                ewsT = pb.tile([128, 4, H], dt.bfloat16,
                                               tag="ewsT", bufs=5)
                                ewsTs[bi] = ewsT
                            else:
                                ewsT = ewsTs[bi]
                            wvg = pb.tile([128, 4, D], dt.bfloat16,
                                          tag="wvg", bufs=4)
                            for tp in range(0, bt, 2):
                                pt = min(2, bt - tp)
                                if "qbshare" in opts:
                                    bg = psqg.tile([128, 2, D], dt.bfloat16,
                                                   tag="qg")
                                else:
                                    bg = psb2.tile([128, 2, D], dt.bfloat16,
                                                   tag="bg")
                                for t2 in range(pt):
                                    t = tp + t2
                                    nc.tensor.matmul(
                                        bg[:, t2, :],
                                        ewss[bi][:, t * ET : (t + 1) * ET],
                                        m1[:], is_transpose=True,
                                    )
                                    if "ewstbg" in opts:
                                        nc.scalar.copy(
                                            ewsT[:, t, :], bg[:, t2, :: HD]
                                        )
                                nc.vector.tensor_tensor(
                                    wvg[:, tp : tp + pt, :],
                                    vG[:, b0 + tp : b0 + tp + pt, :],
                                    bg[:, :pt, :],
                                    Alu.mult,
                                )
                            wvgs[bi] = wvg
                        for bi, (hf, b0, bt, t0) in enumerate(blocks):
                            for t in range(bt):
                                tt = t0 + b0 + t
                                nc.tensor.matmul(
                                    unnorm[:], sgw[:, tt, :],
                                    wvgs[bi][:, t, :],
                                    start=(tt == 0), stop=(tt == T - 1),
                                )
                                nc.tensor.matmul(
                                    den[:], sgw[:, tt, :],
                                    ewsTs[bi][:, t, :],
                                    start=(tt == 0), stop=(tt == T - 1),
                                )
                    else:
                        for hf, b0, bt, t0 in blocks:
                            kT, vG = halves[hf]
                            EB = bt * ET
                            ecol = (t0 + b0) * ET  # within-window edge col
                            # Q broadcast into bf16 PSUM (transpose-mode
                            # selection matmul)
                            kq = pb.tile([128, 4, 512], dt.bfloat16, tag="kq")
                            if "qg4" in opts:
                                qg = psqg.tile(
                                    [128, 4, 512], dt.bfloat16, tag="qg"
                                )
                                for c in range(4):
                                    nc.tensor.matmul(
                                        qg[:, c, :EB],
                                        q_sb[:, w, c * 128 : (c + 1) * 128],
                                        stw[:, ecol : ecol + EB],
                                        is_transpose=True,
                                    )
                                # KQ elementwise, one op per block (DVE,
                                # PSUM-direct, all-bf16)
                                nc.vector.tensor_tensor(
                                    kq[:, :, :EB],
                                    kT[:, :, b0 * ET : b0 * ET + EB],
                                    qg[:, :, :EB],
                                    Alu.mult,
                                )
                            else:
                                for cp in range(2):
                                    qg = psqg.tile(
                                        [128, 2, 512], dt.bfloat16, tag="qg"
                                    )
                                    for c2 in range(2):
                                        c = cp * 2 + c2
                                        nc.tensor.matmul(
                                            qg[:, c2, :EB],
                                            q_sb[:, w, c * 128 : (c + 1) * 128],
                                            stw[:, ecol : ecol + EB],
                                            is_transpose=True,
                                        )
                                    # KQ elementwise (DVE, PSUM-direct)
                                    nc.vector.tensor_tensor(
                                        kq[:, 2 * cp : 2 * cp + 2, :EB],
                                        kT[:, 2 * cp : 2 * cp + 2,
                                           b0 * ET : b0 * ET + EB],
                                        qg[:, :, :EB],
                                        Alu.mult,
                                    )
                            # per-head score reduce (PE)
                            if "scmlp" in opts:
                                sc_ps = pssc.tile(
                                    [128, 512], dt.float32, tag="sc"
                                )
                            else:
                                sc_ps = pssc.tile([H, 512], dt.float32, tag="sc")
                            for c in range(4):
                                nc.tensor.matmul(
                                    sc_ps[:H, :EB], hmask[:, c, :], kq[:, c, :EB],
                                    start=(c == 0), stop=(c == 3),
                                )
                            # ws = scores * ew (DVE, PSUM-direct), f32
                            ws = pb.tile([H, 512], dt.float32, tag="ws")
                            nc.vector.tensor_tensor(
                                ws[:, :EB], sc_ps[:H, :EB],
                                ewtw[:, ecol : ecol + EB], Alu.mult,
                            )
                            # heads-layout exp (ACT) for the b broadcast lhsT
                            ews = pb.tile([H, 512], dt.bfloat16, tag="ews")
                            nc.scalar.activation(ews[:, :EB], ws[:, :EB], Act.Exp)

                            ewsT = pb.tile([128, 4, H], dt.bfloat16, tag="ewsT")
                            if "ewstbg" not in opts:
                                for t in range(bt):
                                    # transposed scores -> exp -> [128e, 8h]
                                    wsT = pswt.tile(
                                        [128, H], dt.float32, tag="wsT"
                                    )
                                    nc.tensor.matmul(
                                        wsT[:], ws[:, t * ET : (t + 1) * ET],
                                        id32[:H, :H], is_transpose=True,
                                    )
                                    nc.scalar.activation(
                                        ewsT[:, t, :], wsT[:], Act.Exp
                                    )
                            wvg = pb.tile([128, 4, D], dt.bfloat16, tag="wvg")
                            for tp in range(0, bt, 2):
                                pt = min(2, bt - tp)
                                # b broadcast (PE K=8, transpose-mode
                                # selection) into bf16 PSUM, tile pairs
                                if "qbshare" in opts:
                                    bg = psqg.tile(
                                        [128, 2, D], dt.bfloat16, tag="qg"
                                    )
                                else:
                                    bg = psb2.tile(
                                        [128, 2, D], dt.bfloat16, tag="bg"
                                    )
                                for t2 in range(pt):
                                    t = tp + t2
                                    nc.tensor.matmul(
                                        bg[:, t2, :],
                                        ews[:, t * ET : (t + 1) * ET],
                                        m1[:], is_transpose=True,
                                    )
                                    if "ewstbg" in opts:
                                        # exp'd transposed scores = strided
                                        # slice of the exp'd broadcast
                                        nc.scalar.copy(
                                            ewsT[:, t, :], bg[:, t2, :: HD]
                                        )
                                # WV (DVE, PSUM-direct, all-bf16)
                                nc.vector.tensor_tensor(
                                    wvg[:, tp : tp + pt, :],
                                    vG[:, b0 + tp : b0 + tp + pt, :],
                                    bg[:, :pt, :],
                                    Alu.mult,
                                )
                                for t2 in range(pt):
                                    t = tp + t2
                                    tt = t0 + b0 + t
                                    nc.tensor.matmul(
                                        unnorm[:], sgw[:, tt, :], wvg[:, t, :],
                                        start=(tt == 0), stop=(tt == T - 1),
                                    )
                                    nc.tensor.matmul(
                                        den[:], sgw[:, tt, :], ewsT[:, t, :],
                                        start=(tt == 0), stop=(tt == T - 1),
                                    )

                    if "epidefer" in opts:
                        if pend is not None:
                            emit_epilogue(*pend)
                        pend = (unnorm, den, w)
                    else:
                        emit_epilogue(unnorm, den, w)
                if pend is not None:
                    emit_epilogue(*pend)

    nc.compile()
    from concourse.bass_interp import get_hw_module

    nc.m = get_hw_module(nc.m)
    return nc


def kernel(x, edge_index, edge_weights, ln_g, ln_b, Wq, bq, Wk, bk, Wv, bv,
           W1, b1, W2, b2, _trace=False):
    x = np.asarray(x, np.float32)
    ei = np.asarray(edge_index)
    ew = np.asarray(edge_weights, np.float32)
    origin, dest = ei[0].astype(np.int64), ei[1].astype(np.int64)

    percore, T, wslot, perm = _host_prep(origin, dest, ew)

    # fold LN affine + attention scale into weights (host, fp32)
    ln_g = np.asarray(ln_g, np.float32)
    ln_b = np.asarray(ln_b, np.float32)
    Wq_f = (ln_g[:, None] * np.asarray(Wq, np.float32)) * SCALE
    bq_f = (ln_b @ np.asarray(Wq, np.float32)) * SCALE + np.asarray(bq, np.float32) * SCALE
    Wk_f = ln_g[:, None] * np.asarray(Wk, np.float32)
    bk_f = ln_b @ np.asarray(Wk, np.float32) + np.asarray(bk, np.float32)
    Wv_f = ln_g[:, None] * np.asarray(Wv, np.float32)
    bv_f = ln_b @ np.asarray(Wv, np.float32) + np.asarray(bv, np.float32)
    W12 = np.asarray(W1, np.float32) @ np.asarray(W2, np.float32)
    b12 = np.asarray(b1, np.float32) @ np.asarray(W2, np.float32) + np.asarray(b2, np.float32)

    def chunked(wm):  # [512, 512] -> [128, 4, 512]
        return np.ascontiguousarray(
            wm.reshape(4, 128, D).transpose(1, 0, 2)
        ).astype(bf16)

    hmask = np.zeros((128, 4, H), np.float32)
    for c in range(4):
        for d in range(128):
            hmask[d, c, (128 * c + d) // HD] = 1.0
    m1 = np.zeros((H, D), np.float32)
    for h in range(H):
        m1[h, h * HD : (h + 1) * HD] = 1.0
    bias_all = np.stack([bq_f, bk_f, bv_f, b12])[None]

    common = dict(
        wq=chunked(Wq_f), wk=chunked(Wk_f), wv=chunked(Wv_f), w12=chunked(W12),
        bias=bias_all.astype(bf16), hmask=hmask.astype(bf16),
        m1=m1.astype(bf16), ones1=np.ones((1, 128), bf16),
        ident=np.eye(128, dtype=bf16),
        id32=np.eye(128, dtype=np.float32),
        iotaF=np.tile(np.arange(128, dtype=bf16)[None, :], (128, 1)),
    )

    in_maps = []
    for r in range(NCORES):
        pc = percore[r]
        xs = np.zeros((ROWS, D), np.float32)
        pr = perm[r * ROWS : (r + 1) * ROWS]
        real = pr >= 0
        xs[real] = x[pr[real]]
        in_maps.append(dict(
            xs=xs,
            dkw=_wrap_idx(pc["dk"].reshape(-1)),
            oloc=np.ascontiguousarray(
                pc["ol"].reshape(W * T, ET).T).astype(np.float32),
            st=np.ascontiguousarray(
                pc["st"].transpose(1, 0, 2).reshape(WIN, -1)).astype(bf16),
            ewt=np.ascontiguousarray(
                pc["et"].reshape(-1, H).T).astype(bf16),
            **common,
        ))

    nc = _build_program(T)
    from concourse import bass_utils

    try:
        res = bass_utils.run_bass_kernel_spmd(
            nc, in_maps, core_ids=list(range(NCORES)), trace=bool(_trace)
        )
    except (ImportError, ModuleNotFoundError):
        # NTFF profiling hook unavailable in this container
        res = bass_utils.run_bass_kernel_spmd(
            nc, in_maps, core_ids=list(range(NCORES))
        )
    out = np.zeros((N, D), np.float32)
    for r in range(NCORES):
        pr = perm[r * ROWS : (r + 1) * ROWS]
        real = pr >= 0
        out[pr[real]] = np.asarray(res.results[r]["out"], np.float32)[real]
    kernel.last_result = res
    if _trace:
        if res.exec_time_ns is not None:
            kernel.exec_time_ns = res.exec_time_ns
        else:
            kernel.exec_time_ns = _bench_pjrt(nc, in_maps)
    return out


def _bench_pjrt(nc, in_maps, iters=4):
    """Re-run the compiled NEFF with device-resident inputs; min wall time."""
    import time
    import jax
    import jax.numpy as jnp
    from jax.sharding import Mesh, PartitionSpec
    from jax.experimental.shard_map import shard_map
    import concourse.mybir as mybir
    from concourse import bass2jax
    from concourse.bass2jax import _bass_exec_p

    bass2jax.install_neuronx_cc_hook()
    partition_name = nc.partition_id_tensor.name if nc.partition_id_tensor else None
    in_names, out_names, out_avals = [], [], []
    for alloc in nc.m.functions[0].allocations:
        if not isinstance(alloc, mybir.MemoryLocationSet):
            continue
        name = alloc.memorylocations[0].name
        if alloc.kind == "ExternalInput":
            if name != partition_name:
                in_names.append(name)
        elif alloc.kind == "ExternalOutput":
            out_names.append(name)
            out_avals.append(
                jax.core.ShapedArray(tuple(alloc.tensor_shape), mybir.dt.np(alloc.dtype))
            )
    n_params = len(in_names)
    all_names = in_names + out_names
    if partition_name is not None:
        all_names.append(partition_name)

    def _body(*args):
        operands = list(args)
        if partition_name is not None:
            operands.append(bass2jax.partition_id_tensor())
        return tuple(_bass_exec_p.bind(
            *operands, out_avals=tuple(out_avals), in_names=tuple(all_names),
            out_names=tuple(out_names), lowering_input_output_aliases=(),
            sim_require_finite=True, sim_require_nnan=True, nc=nc,
        ))

    devices = jax.devices()[:NCORES]
    mesh = Mesh(np.array(devices), ("core",))
    nin = n_params + len(out_names)
    fn = jax.jit(shard_map(_body, mesh=mesh, in_specs=(PartitionSpec("core"),) * nin,
                           out_specs=(PartitionSpec("core"),) * len(out_names),
                           check_rep=False), keep_unused=True)
    concat_in = [
        jnp.concatenate([jnp.asarray(np.asarray(in_maps[c][nm])) for c in range(NCORES)], axis=0)
        for nm in in_names
    ]
    zeros = [jnp.zeros((NCORES * a.shape[0], *a.shape[1:]), a.dtype) for a in out_avals]
    sharding = jax.sharding.NamedSharding(mesh, PartitionSpec("core"))
    concat_in = [jax.device_put(a, sharding) for a in concat_in]
    zeros = [jax.device_put(z, sharding) for z in zeros]
    outs = fn(*concat_in, *zeros)
    jax.block_until_ready(outs)
    best = float("inf")
    for _ in range(iters):
        t0 = time.perf_counter()
        outs = fn(*concat_in, *zeros)
        jax.block_until_ready(outs)
        best = min(best, time.perf_counter() - t0)
    return best * 1e9
